# revision 1
# baseline (speedup 1.0000x reference)
"""Trainium2 Bass kernel for nn_Decompressor (LSTM decompressor).

Reference computation (see problem statement):
    T=256 steps of an LSTM (batch B=128, hidden P=1024) whose output feeds
    back as its input, followed by a linear projection to E=1024:
        gates_t = xin @ W_ih.T + h @ W_hh.T + (b_ih + b_hh)
        i,f,g,o = split(gates_t); c = sig(f)*c + sig(i)*tanh(g)
        h = sig(o)*tanh(c);  xin_{t+1} = h
        out[:, t, :] = h_t @ W_out.T + b_out
    Since xin == h for t>=1, gates_t = h @ (W_ih + W_hh).T + b for t>=1.

Distribution (8 NeuronCores, SPMD):
    Model-parallel over the hidden dimension: core j owns hidden units
    [128j, 128j+128). Each step, core j computes its 512 gate columns
    (i|f|o|g of its units) as a full-batch matmul (M=B=128, K=1024, N=512,
    float32r), applies the LSTM cell pointwise, transposes its h-slice, and
    an AllGather reassembles the full h^T (the next step's stationary
    operand) on every core. The output projection is sharded over E: core j
    computes out^T[Ej, :, :] from the gathered h^T, batching 2 timesteps per
    matmul (N=256) so float32r runs at full rate.

Host side: weight reordering/slicing, x transpose, and final concat/
transpose of the per-core [E_j, T, B] outputs back to [B, T, E].
"""

import numpy as np

import concourse.bacc as bacc
import concourse.mybir as mybir
import concourse.tile as tile
from concourse.bass_utils import run_bass_kernel_spmd

B = 128
P = 1024
E = 1024
T = 256
NC = 8
KT = P // 128          # 8 K-tiles
GS = 4 * P // NC       # 512 gate columns per core
ES = E // NC           # 128 output columns per core

F32 = mybir.dt.float32
F32R = mybir.dt.float32r


def _build(t_steps=T, variant="full"):
    nc = bacc.Bacc("TRN2", target_bir_lowering=False, debug=False, num_devices=NC)

    wrec_d = nc.dram_tensor("wrec", [128, KT * GS], F32R, kind="ExternalInput").ap()
    win_d = nc.dram_tensor("win", [128, KT * GS], F32R, kind="ExternalInput").ap()
    xT_d = nc.dram_tensor("xT", [128, KT * B], F32R, kind="ExternalInput").ap()
    bias_d = nc.dram_tensor("bias", [1, GS], F32R, kind="ExternalInput").ap()
    biasf_d = nc.dram_tensor("biasf", [128, GS], F32, kind="ExternalInput").ap()
    wout_d = nc.dram_tensor("wout", [128, KT * ES], F32R, kind="ExternalInput").ap()
    bout_d = nc.dram_tensor("bout", [128, 1], F32, kind="ExternalInput").ap()
    ones_d = nc.dram_tensor("ones", [1, 128], F32R, kind="ExternalInput").ap()
    ident_d = nc.dram_tensor("ident", [128, 128], F32R, kind="ExternalInput").ap()
    outT_d = nc.dram_tensor("outT", [ES, t_steps, B], F32, kind="ExternalOutput").ap()

    nb = 4 if "b4" in variant else (3 if "b3" in variant else 2)
    with tile.TileContext(nc) as tc:
        with (
            tc.tile_pool(name="const", bufs=1) as cpool,
            tc.tile_pool(name="state", bufs=1) as spool,
            tc.tile_pool(name="work", bufs=nb) as wk,
            tc.tile_pool(name="pair", bufs=nb) as pairp,
            tc.tile_pool(name="psum", bufs=2, space="PSUM") as psg,
            tc.tile_pool(name="dram", bufs=nb + 1, space="DRAM") as dram,
        ):
            wrec_sb = cpool.tile([128, KT * GS], F32R)
            win_sb = cpool.tile([128, KT * GS], F32R)
            xT_sb = cpool.tile([128, KT * B], F32R)
            bias_sb = cpool.tile([1, GS], F32R)
            biasf_sb = cpool.tile([128, GS], F32)
            wout_sb = cpool.tile([128, KT * ES], F32R)
            bout_sb = cpool.tile([128, 1], F32)
            ones_sb = cpool.tile([1, 128], F32R)
            ident_sb = cpool.tile([128, 128], F32R)
            nc.sync.dma_start(wrec_sb[:], wrec_d[:])
            nc.sync.dma_start(win_sb[:], win_d[:])
            nc.sync.dma_start(xT_sb[:], xT_d[:])
            nc.sync.dma_start(bias_sb[:], bias_d[:])
            nc.sync.dma_start(biasf_sb[:], biasf_d[:])
            nc.sync.dma_start(wout_sb[:], wout_d[:])
            nc.sync.dma_start(bout_sb[:], bout_d[:])
            nc.sync.dma_start(ones_sb[:], ones_d[:])
            nc.sync.dma_start(ident_sb[:], ident_d[:])

            c_sb = spool.tile([128, 128], F32)

            pair = None
            for s in range(t_steps):
                # ---- gates matmul: [B, GS] psum, bias via K=1 matmul ----
                ps_g = psg.tile([128, GS], F32, tag="ps_g")
                nobias = "nobias" in variant or "dvb" in variant
                if not nobias:
                    nc.tensor.matmul(ps_g[:], ones_sb[:], bias_sb[:],
                                     start=True, stop=False)
                w_sb = win_sb if s == 0 else wrec_sb
                for k in range(KT):
                    if s == 0:
                        lhsT = xT_sb[:, k * B:(k + 1) * B]
                    else:
                        slot = (s - 1) % 2
                        lhsT = pair[:, slot * P + k * 128: slot * P + (k + 1) * 128]
                    nc.tensor.matmul(ps_g[:], lhsT, w_sb[:, k * GS:(k + 1) * GS],
                                     start=(nobias and k == 0), stop=(k == KT - 1))

                # ---- pointwise: layout [i|f|o|g] each 128 wide ----
                gates_sb = wk.tile([128, GS], F32, tag="gates")
                if "dvb" in variant:
                    gpre = wk.tile([128, GS], F32, tag="gpre")
                    nc.vector.tensor_tensor(gpre[:], ps_g[:], biasf_sb[:],
                                            mybir.AluOpType.add)
                    act_src = gpre
                else:
                    act_src = ps_g
                nc.scalar.activation(gates_sb[:, 0:384], act_src[:, 0:384],
                                     mybir.ActivationFunctionType.Sigmoid)
                nc.scalar.activation(gates_sb[:, 384:512], act_src[:, 384:512],
                                     mybir.ActivationFunctionType.Tanh)
                i_ap = gates_sb[:, 0:128]
                f_ap = gates_sb[:, 128:256]
                o_ap = gates_sb[:, 256:384]
                g_ap = gates_sb[:, 384:512]
                if s == 0:
                    nc.vector.tensor_tensor(c_sb[:], i_ap, g_ap,
                                            mybir.AluOpType.mult)
                else:
                    ig = wk.tile([128, 128], F32, tag="ig")
                    nc.vector.tensor_tensor(ig[:], i_ap, g_ap,
                                            mybir.AluOpType.mult)
                    nc.vector.tensor_tensor(c_sb[:], c_sb[:], f_ap,
                                            mybir.AluOpType.mult)
                    nc.vector.tensor_tensor(c_sb[:], c_sb[:], ig[:],
                                            mybir.AluOpType.add)
                th = wk.tile([128, 128], F32, tag="th")
                nc.scalar.activation(th[:], c_sb[:],
                                     mybir.ActivationFunctionType.Tanh)
                h_sb = wk.tile([128, 128], F32R, tag="h")
                nc.vector.tensor_tensor(h_sb[:], o_ap, th[:],
                                        mybir.AluOpType.mult)

                # ---- transpose h slice -> [units, B] and exchange ----
                ps_t = psg.tile([128, 128], F32R, tag="ps_t")
                nc.tensor.transpose(ps_t[:], h_sb[:], ident_sb[:])
                hT_send = wk.tile([128, 128], F32R, tag="hTs")
                if "acp" in variant:
                    nc.scalar.copy(hT_send[:], ps_t[:])
                else:
                    nc.vector.tensor_copy(hT_send[:], ps_t[:])

                if "sbex" in variant:
                    if s % 2 == 0:
                        pair = pairp.tile([128, 2 * P], F32R, tag="pair")
                    slot = s % 2
                    for k in range(KT):
                        nc.vector.tensor_copy(
                            pair[:, slot * P + k * 128: slot * P + (k + 1) * 128],
                            hT_send[:])
                else:
                    inb = dram.tile([128, B], F32R, tag="inb")
                    outb = dram.tile([NC * 128, B], F32R, tag="outb")
                    (nc.scalar if "sdma" in variant else nc.sync).dma_start(inb[:], hT_send[:])
                if "sbex" in variant:
                    pass
                elif "noag" not in variant:
                    nc.gpsimd.collective_compute(
                        "AllGather", mybir.AluOpType.bypass,
                        ins=[inb.opt()], outs=[outb.opt()],
                        replica_groups=[list(range(NC))],
                    )
                if "sbex" in variant:
                    pass
                elif True:
                    if s % 2 == 0:
                        pair = pairp.tile([128, 2 * P], F32R, tag="pair")
                slot = s % 2
                if "sbex" in variant:
                    pass
                elif "noag" in variant:
                    for k in range(KT):
                        nc.sync.dma_start(
                            pair[:, slot * P + k * 128: slot * P + (k + 1) * 128],
                            inb.opt(),
                        )
                else:
                    src3 = outb.opt().rearrange("(k p) b -> p k b", p=128)
                    for k in range(KT):
                        eng = nc.scalar if ("sdma" in variant and k % 2 == 1) else nc.sync
                        eng.dma_start(
                            pair[:, slot * P + k * 128: slot * P + (k + 1) * 128],
                            src3[:, k, :],
                        )

                # ---- projection of steps (s-1, s) every odd step ----
                if "noproj" in variant:
                    continue
                if s % 2 == 1 or s == t_steps - 1:
                    if s % 2 == 1:
                        nsteps, base = 2, s - 1
                    else:  # odd t_steps tail: project the single last step
                        nsteps, base = 1, s
                    ps_p = psg.tile([128, nsteps * B], F32, tag="ps_p")
                    rhs4 = pair[:].rearrange("p (s2 k b) -> p s2 k b", s2=2, k=KT)
                    for k in range(KT):
                        rhs = rhs4[:, 0:nsteps, k, :]
                        nc.tensor.matmul(ps_p[:], wout_sb[:, k * ES:(k + 1) * ES],
                                         rhs, start=(k == 0), stop=(k == KT - 1))
                    out_sb = wk.tile([128, nsteps * B], F32, tag="out_sb")
                    nc.scalar.activation(out_sb[:], ps_p[:],
                                         mybir.ActivationFunctionType.Identity,
                                         bias=bout_sb[:, 0:1])
                    dst = outT_d[:, base:base + nsteps, :]
                    nc.sync.dma_start(
                        dst, out_sb[:].rearrange("p (s2 b) -> p s2 b", s2=nsteps))

    nc.compile()
    return nc


def _prep_inputs(x, W_ih, W_hh, b_ih, b_hh, W_out, b_out):
    x = np.asarray(x, np.float32)
    W_ih = np.asarray(W_ih, np.float32)
    W_hh = np.asarray(W_hh, np.float32)
    b_ih = np.asarray(b_ih, np.float32)
    b_hh = np.asarray(b_hh, np.float32)
    W_out = np.asarray(W_out, np.float32)
    b_out = np.asarray(b_out, np.float32)

    Wsum = W_ih + W_hh
    bsum = b_ih + b_hh
    # xT as [8 K-tiles, 128, B] laid out [128, KT*B] per tile columns:
    xT_tiles = np.ascontiguousarray(x.T).reshape(KT, 128, B)
    xT_flat = np.concatenate([xT_tiles[k] for k in range(KT)], axis=1)

    ones = np.ones((1, 128), np.float32)
    ident = np.eye(128, dtype=np.float32)

    in_maps = []
    for j in range(NC):
        js = np.arange(128 * j, 128 * (j + 1))
        rows = np.concatenate([0 * P + js, 1 * P + js, 3 * P + js, 2 * P + js])
        Wj = Wsum[rows, :]                       # [512, 1024]
        wrec_t = np.ascontiguousarray(Wj.T).reshape(KT, 128, GS)
        wrec = np.concatenate([wrec_t[k] for k in range(KT)], axis=1)
        Wji = W_ih[rows, :]
        win_t = np.ascontiguousarray(Wji.T).reshape(KT, 128, GS)
        win = np.concatenate([win_t[k] for k in range(KT)], axis=1)
        bias = bsum[rows][None, :]
        Woj = W_out[128 * j:128 * (j + 1), :]    # [128, 1024]
        wout_t = np.ascontiguousarray(Woj.T).reshape(KT, 128, ES)
        wout = np.concatenate([wout_t[k] for k in range(KT)], axis=1)
        bout = b_out[128 * j:128 * (j + 1)][:, None]
        in_maps.append({
            "biasf": np.broadcast_to(bias, (128, GS)).copy(),
            "wrec": np.ascontiguousarray(wrec),
            "win": np.ascontiguousarray(win),
            "xT": np.ascontiguousarray(xT_flat),
            "bias": np.ascontiguousarray(bias),
            "wout": np.ascontiguousarray(wout),
            "bout": np.ascontiguousarray(bout),
            "ones": ones,
            "ident": ident,
        })
    return in_maps


_NC_CACHE = {}


def kernel_mp8(x, W_ih, W_hh, b_ih, b_hh, W_out, b_out, _t_steps=T):
    if _t_steps not in _NC_CACHE:
        _NC_CACHE[_t_steps] = _build(_t_steps)
    nc = _NC_CACHE[_t_steps]
    in_maps = _prep_inputs(x, W_ih, W_hh, b_ih, b_hh, W_out, b_out)
    res = run_bass_kernel_spmd(nc, in_maps, list(range(NC)))
    parts = [res.results[j]["outT"] for j in range(NC)]   # each [ES, T, B]
    full = np.concatenate(parts, axis=0)                  # [E, T, B]
    return np.ascontiguousarray(full.transpose(2, 1, 0))  # [B, T, E]


# ---------------- 2x4 hybrid: batch halves x 4-way unit shard ----------------
# cores {0-3}: batch rows 0:64, cores {4-7}: rows 64:128 (replica groups
# [[0,1,2,3],[4,5,6,7]] run their AllGathers CONCURRENTLY). Each core owns 256
# hidden units as two 128-unit blocks (a/b); per K-tile one LDW serves both
# blocks' matmuls. Projection: 256 E-cols per core, batched over 4 steps
# (N=4*64=256) from a "quad" buffer.
BL = 64           # local batch
UB = 2            # unit blocks per core


def _build_h24(t_steps=T):
    nc = bacc.Bacc("TRN2", target_bir_lowering=False, debug=False, num_devices=NC)
    GS2 = 1024    # gate cols per core (2 blocks x 512)
    ES2 = 256     # E cols per core

    wrec_d = nc.dram_tensor("wrec", [128, KT * GS2], F32R, kind="ExternalInput").ap()
    win_d = nc.dram_tensor("win", [128, KT * GS2], F32R, kind="ExternalInput").ap()
    xT_d = nc.dram_tensor("xT", [128, KT * BL], F32R, kind="ExternalInput").ap()
    bias_d = nc.dram_tensor("bias", [1, GS2], F32R, kind="ExternalInput").ap()
    wout_d = nc.dram_tensor("wout", [128, KT * ES2], F32R, kind="ExternalInput").ap()
    bout_d = nc.dram_tensor("bout", [128, UB], F32, kind="ExternalInput").ap()
    ones_d = nc.dram_tensor("ones", [1, BL], F32R, kind="ExternalInput").ap()
    ident_d = nc.dram_tensor("ident", [128, 128], F32R, kind="ExternalInput").ap()
    outT_d = nc.dram_tensor("outT", [ES2, t_steps, BL], F32, kind="ExternalOutput").ap()

    with tile.TileContext(nc) as tc:
        with (
            tc.tile_pool(name="const", bufs=1) as cpool,
            tc.tile_pool(name="state", bufs=1) as spool,
            tc.tile_pool(name="work", bufs=2) as wk,
            tc.tile_pool(name="quad", bufs=2) as quadp,
            tc.tile_pool(name="psum", bufs=2, space="PSUM") as psg,
            tc.tile_pool(name="dram", bufs=3, space="DRAM") as dram,
        ):
            wrec_sb = cpool.tile([128, KT * GS2], F32R)
            win_sb = cpool.tile([128, KT * GS2], F32R)
            xT_sb = cpool.tile([128, KT * BL], F32R)
            bias_sb = cpool.tile([1, GS2], F32R)
            wout_sb = cpool.tile([128, KT * ES2], F32R)
            bout_sb = cpool.tile([128, UB], F32)
            ones_sb = cpool.tile([1, BL], F32R)
            ident_sb = cpool.tile([128, 128], F32R)
            nc.sync.dma_start(wrec_sb[:], wrec_d[:])
            nc.sync.dma_start(win_sb[:], win_d[:])
            nc.sync.dma_start(xT_sb[:], xT_d[:])
            nc.sync.dma_start(bias_sb[:], bias_d[:])
            nc.sync.dma_start(wout_sb[:], wout_d[:])
            nc.sync.dma_start(bout_sb[:], bout_d[:])
            nc.sync.dma_start(ones_sb[:], ones_d[:])
            nc.sync.dma_start(ident_sb[:], ident_d[:])

            c_sb = spool.tile([128, 2 * 128], F32)   # [64used, block v cols]

            quad = None
            SLOTW = KT * BL            # 512 cols per step-slot in quad
            for s in range(t_steps):
                # gates for both unit blocks; one K-tile LDW serves both MMs
                pss = []
                for v in range(UB):
                    ps_v = psg.tile([128, GS], F32, tag=f"ps_g{v}")
                    nc.tensor.matmul(ps_v[0:BL, :], ones_sb[:],
                                     bias_sb[:, v * GS:(v + 1) * GS],
                                     start=True, stop=False)
                    pss.append(ps_v)
                w_sb = win_sb if s == 0 else wrec_sb
                for k in range(KT):
                    if s == 0:
                        lhsT = xT_sb[:, k * BL:(k + 1) * BL]
                    else:
                        slot = (s - 1) % 4
                        lhsT = quad_prev[:, slot * SLOTW + k * BL: slot * SLOTW + (k + 1) * BL]
                    for v in range(UB):
                        nc.tensor.matmul(
                            pss[v][0:BL, :], lhsT,
                            w_sb[:, (k * UB + v) * GS:(k * UB + v + 1) * GS],
                            start=False, stop=(k == KT - 1))

                ps_t = psg.tile([128, 128], F32R, tag="ps_t")
                for v in range(UB):
                    gates_sb = wk.tile([BL, GS], F32, tag=f"gates{v}")
                    nc.scalar.activation(gates_sb[:, 0:384], pss[v][0:BL, 0:384],
                                         mybir.ActivationFunctionType.Sigmoid)
                    nc.scalar.activation(gates_sb[:, 384:512], pss[v][0:BL, 384:512],
                                         mybir.ActivationFunctionType.Tanh)
                    i_ap = gates_sb[:, 0:128]
                    f_ap = gates_sb[:, 128:256]
                    o_ap = gates_sb[:, 256:384]
                    g_ap = gates_sb[:, 384:512]
                    c_v = c_sb[0:BL, v * 128:(v + 1) * 128]
                    if s == 0:
                        nc.vector.tensor_tensor(c_v, i_ap, g_ap, mybir.AluOpType.mult)
                    else:
                        ig = wk.tile([BL, 128], F32, tag=f"ig{v}")
                        nc.vector.tensor_tensor(ig[:], i_ap, g_ap, mybir.AluOpType.mult)
                        nc.vector.tensor_tensor(c_v, c_v, f_ap, mybir.AluOpType.mult)
                        nc.vector.tensor_tensor(c_v, c_v, ig[:], mybir.AluOpType.add)
                    th = wk.tile([BL, 128], F32, tag=f"th{v}")
                    nc.scalar.activation(th[:], c_v, mybir.ActivationFunctionType.Tanh)
                    h_sb = wk.tile([BL, 128], F32R, tag=f"h{v}")
                    nc.vector.tensor_tensor(h_sb[:], o_ap, th[:], mybir.AluOpType.mult)
                    # transpose block v: [64,128] -> [128,64] at ps_t cols v*64
                    nc.tensor.transpose(ps_t[:, v * BL:(v + 1) * BL], h_sb[:],
                                        ident_sb[0:BL, 0:BL])
                hT_send = wk.tile([128, UB * BL], F32R, tag="hTs")
                nc.vector.tensor_copy(hT_send[:], ps_t[:])

                inb = dram.tile([UB * 128, BL], F32R, tag="inb")
                outb = dram.tile([NC // 2 * UB * 128, BL], F32R, tag="outb")
                nc.sync.dma_start(
                    inb.opt().rearrange("(v p) b -> p v b", p=128),
                    hT_send[:].rearrange("p (v b) -> p v b", v=UB))
                nc.gpsimd.collective_compute(
                    "AllGather", mybir.AluOpType.bypass,
                    ins=[inb.opt()], outs=[outb.opt()],
                    replica_groups=[[0, 1, 2, 3], [4, 5, 6, 7]],
                )
                if s % 4 == 0:
                    quad = quadp.tile([128, 4 * SLOTW], F32R, tag="quad")
                slot = s % 4
                src3 = outb.opt().rearrange("(k p) b -> p k b", p=128)
                for k in range(KT):
                    nc.sync.dma_start(
                        quad[:, slot * SLOTW + k * BL: slot * SLOTW + (k + 1) * BL],
                        src3[:, k, :])
                quad_prev = quad

                # projection of steps (s-3..s) every 4th step
                if s % 4 == 3 or s == t_steps - 1:
                    nst = 4 if s % 4 == 3 else (s % 4) + 1
                    base = s - nst + 1
                    rhs5 = quad[:].rearrange("p (s4 k b) -> p s4 k b", s4=4, k=KT)
                    for m in range(UB):
                        ps_p = psg.tile([128, nst * BL], F32, tag="ps_p")
                        for k in range(KT):
                            nc.tensor.matmul(
                                ps_p[:], wout_sb[:, (k * UB + m) * 128:(k * UB + m + 1) * 128],
                                rhs5[:, 0:nst, k, :],
                                start=(k == 0), stop=(k == KT - 1))
                        out_sb = wk.tile([128, nst * BL], F32, tag="out_sb")
                        nc.scalar.activation(out_sb[:], ps_p[:],
                                             mybir.ActivationFunctionType.Identity,
                                             bias=bout_sb[:, m:m + 1])
                        nc.sync.dma_start(
                            outT_d[m * 128:(m + 1) * 128, base:base + nst, :],
                            out_sb[:].rearrange("p (s4 b) -> p s4 b", s4=nst))

    nc.compile()
    return nc


def _prep_inputs_h24(x, W_ih, W_hh, b_ih, b_hh, W_out, b_out):
    x = np.asarray(x, np.float32); W_ih = np.asarray(W_ih, np.float32)
    W_hh = np.asarray(W_hh, np.float32); b_ih = np.asarray(b_ih, np.float32)
    b_hh = np.asarray(b_hh, np.float32); W_out = np.asarray(W_out, np.float32)
    b_out = np.asarray(b_out, np.float32)
    Wsum = W_ih + W_hh
    bsum = b_ih + b_hh
    ones = np.ones((1, BL), np.float32)
    ident = np.eye(128, dtype=np.float32)
    GS2, ES2 = 1024, 256
    in_maps = []
    for j in range(NC):
        half, u = j // 4, j % 4
        xh = x[64 * half:64 * (half + 1), :]                     # [64, 1024]
        xT_t = np.ascontiguousarray(xh.T).reshape(KT, 128, BL)
        xT = np.concatenate([xT_t[k] for k in range(KT)], axis=1)
        # unit blocks a/b: units [256u, 256u+128), [256u+128, 256u+256)
        rows = []
        for v in range(UB):
            js = np.arange(256 * u + 128 * v, 256 * u + 128 * (v + 1))
            rows.append(np.concatenate([0 * P + js, 1 * P + js, 3 * P + js, 2 * P + js]))

        def ktiles(Wmat):
            # per K-tile k: [block_a 512 | block_b 512]
            cols = []
            for k in range(KT):
                seg = [np.ascontiguousarray(Wmat[r, 128 * k:128 * (k + 1)].T)
                       for r in rows]          # each [128, 512]
                cols.append(np.concatenate(seg, axis=1))
            return np.concatenate(cols, axis=1)

        wrec = ktiles(Wsum)
        win = ktiles(W_ih)
        bias = np.concatenate([bsum[r] for r in rows])[None, :]
        # wout: E block = [256j%4... E cols 256u2 where u2 = u (any fixed map)
        ecols = np.arange(256 * u, 256 * (u + 1))
        wo = []
        for k in range(KT):
            seg = [np.ascontiguousarray(
                W_out[ecols[128 * m:128 * (m + 1)], 128 * k:128 * (k + 1)].T)
                for m in range(UB)]           # each [128, 128]
            wo.append(np.concatenate(seg, axis=1))
        wout = np.concatenate(wo, axis=1)
        bout = np.stack([b_out[ecols[128 * m:128 * (m + 1)]] for m in range(UB)], axis=1)
        in_maps.append({
            "wrec": np.ascontiguousarray(wrec), "win": np.ascontiguousarray(win),
            "xT": np.ascontiguousarray(xT), "bias": np.ascontiguousarray(bias),
            "wout": np.ascontiguousarray(wout), "bout": np.ascontiguousarray(bout),
            "ones": ones, "ident": ident,
        })
    return in_maps




# One PJRT executable per built module, reused across kernel() calls.
# run_bass_kernel_spmd builds a fresh executable every call; alternating
# executables that contain collectives within one process can desync the
# mesh (observed), so execute through a single cached callable instead.
_RUNNER_CACHE = {}


class _CachedSpmdRunner:
    def __init__(self, nc):
        import jax
        from jax.sharding import Mesh, PartitionSpec
        from jax.experimental.shard_map import shard_map
        from concourse.bass2jax import (
            _bass_exec_p, install_neuronx_cc_hook, partition_id_tensor)

        install_neuronx_cc_hook()
        partition_name = (
            nc.partition_id_tensor.name if nc.partition_id_tensor else None)
        in_names, out_names, out_avals, zero_outs = [], [], [], []
        for alloc in nc.m.functions[0].allocations:
            if not isinstance(alloc, mybir.MemoryLocationSet):
                continue
            name = alloc.memorylocations[0].name
            if alloc.kind == "ExternalInput":
                if name != partition_name:
                    in_names.append(name)
            elif alloc.kind == "ExternalOutput":
                out_names.append(name)
                shape = tuple(alloc.tensor_shape)
                dtype = mybir.dt.np(alloc.dtype)
                out_avals.append(jax.core.ShapedArray(shape, dtype))
                zero_outs.append(np.zeros(shape, dtype))
        n_params, n_outs = len(in_names), len(out_avals)
        all_in = list(in_names) + list(out_names)
        if partition_name is not None:
            all_in.append(partition_name)

        def _body(*args):
            operands = list(args)
            if partition_name is not None:
                operands.append(partition_id_tensor())
            return tuple(_bass_exec_p.bind(
                *operands, out_avals=tuple(out_avals),
                in_names=tuple(all_in), out_names=tuple(out_names),
                lowering_input_output_aliases=(),
                sim_require_finite=True, sim_require_nnan=True, nc=nc))

        devices = jax.devices()[:NC]
        mesh = Mesh(np.asarray(devices), ("core",))
        self._fn = jax.jit(
            shard_map(_body, mesh=mesh,
                      in_specs=(PartitionSpec("core"),) * (n_params + n_outs),
                      out_specs=(PartitionSpec("core"),) * n_outs,
                      check_rep=False),
            keep_unused=True)
        self._jax = jax
        self.in_names, self.out_names, self.out_avals = in_names, out_names, out_avals
        self._zeros = [
            np.zeros((NC * z.shape[0], *z.shape[1:]), z.dtype) for z in zero_outs]

    def run(self, in_maps):
        per_core = [[np.asarray(m[n]) for n in self.in_names] for m in in_maps]
        concat_in = [
            np.concatenate([per_core[c][i] for c in range(NC)], axis=0)
            for i in range(len(self.in_names))]
        out_arrs = self._fn(*concat_in, *self._zeros)
        self._jax.block_until_ready(out_arrs)
        return [
            {n: np.asarray(out_arrs[i]).reshape(NC, *self.out_avals[i].shape)[c]
             for i, n in enumerate(self.out_names)}
            for c in range(NC)]


def kernel_h24(x, W_ih, W_hh, b_ih, b_hh, W_out, b_out, _t_steps=T):
    key = ("h24", _t_steps)
    if key not in _NC_CACHE:
        _NC_CACHE[key] = _build_h24(_t_steps)
    if key not in _RUNNER_CACHE:
        _RUNNER_CACHE[key] = _CachedSpmdRunner(_NC_CACHE[key])
    in_maps = _prep_inputs_h24(x, W_ih, W_hh, b_ih, b_hh, W_out, b_out)
    results = _RUNNER_CACHE[key].run(in_maps)
    out = np.zeros((128, _t_steps, 1024), np.float32)
    for j in range(NC):
        half, u = j // 4, j % 4
        part = results[j]["outT"]            # [256 E, T, 64 B]
        out[64 * half:64 * (half + 1), :, 256 * u:256 * (u + 1)] = part.transpose(2, 1, 0)
    return out


# ---------------- replicated LSTM, E-sharded projection ----------------
# Every core runs the full recurrence (no cross-core traffic at all): the
# gates matmul is N-bound on the PE (64 matmuls of [128K,128]x[128,512]
# fp32r at 1 cycle/row), so sharding batch would not reduce per-step time,
# and per-step collectives measured 12-54us -- far above the 13.6us PE
# floor. Only the output projection is sharded (core j owns E columns
# [128j, 128j+128)), batched 2 timesteps per matmul (N=256, full fp32r
# rate). Step 0's pre-activation gates (x @ W_ih.T + b) come from the
# host, so W_ih/x never touch the device. Bias is added by the Pool
# engine (PSUM + broadcast bias -> SBUF), keeping the PE at its floor.
# h^T lives in a 4-parity ring so the 2-step projection window is a
# contiguous slice; gate columns are host-permuted to [i|f|o|g] per
# 128-unit block so each 512-col PSUM chunk is one complete block.

def _build_rep(t_steps=T, variant=""):
    nc = bacc.Bacc("TRN2", target_bir_lowering=False, debug=False,
                   num_devices=NC)
    GALL = 4 * P                    # 4096 gate cols
    wsum_d = nc.dram_tensor("wsum", [128, KT * GALL], F32R,
                            kind="ExternalInput").ap()
    biasf_d = nc.dram_tensor("biasf", [128, GALL], F32,
                             kind="ExternalInput").ap()
    g0_d = nc.dram_tensor("g0", [128, GALL], F32, kind="ExternalInput").ap()
    wout_d = nc.dram_tensor("wout", [128, KT * 128], F32R,
                            kind="ExternalInput").ap()
    bout_d = nc.dram_tensor("bout", [128, 1], F32, kind="ExternalInput").ap()
    ident_d = nc.dram_tensor("ident", [128, 128], F32R,
                             kind="ExternalInput").ap()
    outT_d = nc.dram_tensor("outT", [128, t_steps, B], F32,
                            kind="ExternalOutput").ap()

    CW = 1024 if "mm1024" in variant else 512   # psum chunk width
    NCH = GALL // CW                            # chunks per step
    Q = CW // 4                                 # units per chunk
    NSUB = Q // 128                             # 128-unit sub-blocks
    with tile.TileContext(nc) as tc:
        with (
            tc.tile_pool(name="const", bufs=1) as cpool,
            tc.tile_pool(name="state", bufs=1) as spool,
            tc.tile_pool(name="work", bufs=3) as wk,
            tc.tile_pool(name="psg", bufs=(2 if CW == 1024 else 3),
                         space="PSUM") as psg,
            tc.tile_pool(name="pst", bufs=2, space="PSUM") as pst,
            tc.tile_pool(name="psp", bufs=2, space="PSUM") as psp,
        ):
            wsum_sb = cpool.tile([128, KT * GALL], F32R)
            biasf_sb = cpool.tile([128, GALL], F32)
            wout_sb = cpool.tile([128, KT * 128], F32R)
            bout_sb = cpool.tile([128, 1], F32)
            ident_sb = cpool.tile([128, 128], F32R)
            nc.sync.dma_start(wsum_sb[:], wsum_d[:])
            nc.sync.dma_start(biasf_sb[:], biasf_d[:])
            nc.sync.dma_start(wout_sb[:], wout_d[:])
            nc.sync.dma_start(bout_sb[:], bout_d[:])
            nc.sync.dma_start(ident_sb[:], ident_d[:])

            hT = spool.tile([128, 4 * P], F32R)      # 4 step-parities
            c_sb = spool.tile([128, P], F32)

            for s in range(t_steps):
                par = s % 4
                prev = (s - 1) % 4
                for ch in range(NCH):
                    if s == 0:
                        pre = wk.tile([128, CW], F32, tag="pre")
                        nc.sync.dma_start(pre[:],
                                          g0_d[:, ch * CW:(ch + 1) * CW])
                    else:
                        ps_g = psg.tile([128, CW], F32, tag="ps_g")
                        for k in range(KT):
                            nc.tensor.matmul(
                                ps_g[:],
                                hT[:, k * 512 + prev * 128:
                                   k * 512 + (prev + 1) * 128],
                                wsum_sb[:, (k * NCH + ch) * CW:
                                        (k * NCH + ch + 1) * CW],
                                start=(k == 0), stop=(k == KT - 1))
                        pre = wk.tile([128, CW], F32, tag="pre")
                        nc.vector.tensor_tensor(
                            pre[:], ps_g[:],
                            biasf_sb[:, ch * CW:(ch + 1) * CW],
                            mybir.AluOpType.add)
                    gates = wk.tile([128, CW], F32, tag="gates")
                    nc.scalar.activation(gates[:, 0:3 * Q], pre[:, 0:3 * Q],
                                         mybir.ActivationFunctionType.Sigmoid)
                    nc.scalar.activation(gates[:, 3 * Q:CW], pre[:, 3 * Q:CW],
                                         mybir.ActivationFunctionType.Tanh)
                    for w in range(NSUB):
                        i_ap = gates[:, w * 128:(w + 1) * 128]
                        f_ap = gates[:, Q + w * 128:Q + (w + 1) * 128]
                        o_ap = gates[:, 2 * Q + w * 128:2 * Q + (w + 1) * 128]
                        g_ap = gates[:, 3 * Q + w * 128:3 * Q + (w + 1) * 128]
                        u = ch * NSUB + w
                        cu = c_sb[:, u * 128:(u + 1) * 128]
                        if s == 0:
                            nc.vector.tensor_tensor(cu, i_ap, g_ap,
                                                    mybir.AluOpType.mult)
                        else:
                            ig = wk.tile([128, 128], F32, tag="ig")
                            nc.gpsimd.tensor_tensor(ig[:], i_ap, g_ap,
                                                    mybir.AluOpType.mult)
                            nc.vector.tensor_tensor(cu, cu, f_ap,
                                                    mybir.AluOpType.mult)
                            nc.vector.tensor_tensor(cu, cu, ig[:],
                                                    mybir.AluOpType.add)
                        th = wk.tile([128, 128], F32, tag="th")
                        nc.scalar.activation(th[:], cu,
                                             mybir.ActivationFunctionType.Tanh)
                        h_u = wk.tile([128, 128], F32R, tag="h")
                        nc.vector.tensor_tensor(h_u[:], o_ap, th[:],
                                                mybir.AluOpType.mult)
                        if "notrans" not in variant:
                            ps_t = pst.tile([128, 128], F32R, tag="ps_t")
                            nc.tensor.transpose(ps_t[:], h_u[:], ident_sb[:])
                            nc.scalar.copy(
                                hT[:, u * 512 + par * 128:
                                   u * 512 + (par + 1) * 128],
                                ps_t[:])

                if "noproj" in variant:
                    continue
                if s % 4 == 3 or s == t_steps - 1:
                    nsteps = 4 if s % 4 == 3 else (s % 4) + 1
                    base = s - nsteps + 1        # base % 4 == 0
                    # k-major hT layout: parities base..s of k-tile k sit at
                    # [k*512, k*512 + nsteps*128) — one contiguous rhs.
                    ps_p = psp.tile([128, nsteps * B], F32, tag="ps_p")
                    for k in range(KT):
                        nc.tensor.matmul(
                            ps_p[:], wout_sb[:, k * 128:(k + 1) * 128],
                            hT[:, k * 512:k * 512 + nsteps * 128],
                            start=(k == 0), stop=(k == KT - 1))
                    out_sb = wk.tile([128, nsteps * B], F32, tag="out_sb")
                    nc.scalar.activation(out_sb[:], ps_p[:],
                                         mybir.ActivationFunctionType.Identity,
                                         bias=bout_sb[:, 0:1])
                    nc.sync.dma_start(
                        outT_d[:, base:s + 1, :],
                        out_sb[:].rearrange("p (st b) -> p st b", st=nsteps))

    nc.compile()
    return nc


def _prep_inputs_rep(x, W_ih, W_hh, b_ih, b_hh, W_out, b_out, variant=""):
    x = np.asarray(x, np.float32)
    W_ih = np.asarray(W_ih, np.float32)
    W_hh = np.asarray(W_hh, np.float32)
    b_ih = np.asarray(b_ih, np.float32)
    b_hh = np.asarray(b_hh, np.float32)
    W_out = np.asarray(W_out, np.float32)
    b_out = np.asarray(b_out, np.float32)

    Wsum = W_ih + W_hh
    bsum = b_ih + b_hh
    # gate-col permutation: per Q-unit chunk ch, [i_ch | f_ch | o_ch | g_ch]
    # (torch row order is i,f,g,o -> indices 0,1,3,2)
    Q = 256 if "mm1024" in variant else 128
    allrows = np.concatenate([
        np.concatenate([q * P + np.arange(Q * u, Q * (u + 1))
                        for q in (0, 1, 3, 2)])
        for u in range(P // Q)])
    Wperm = Wsum[allrows, :]                       # [4096, 1024]
    wsum = np.concatenate(
        [np.ascontiguousarray(Wperm[:, 128 * k:128 * (k + 1)].T)
         for k in range(KT)], axis=1)              # [128, KT*4096]
    biasf = np.broadcast_to(bsum[allrows], (128, 4 * P)).copy()
    g0 = (x @ W_ih.T + bsum)[:, allrows]           # [128, 4096]
    ident = np.eye(128, dtype=np.float32)

    in_maps = []
    for j in range(NC):
        wout = np.concatenate(
            [np.ascontiguousarray(
                W_out[128 * j:128 * (j + 1), 128 * k:128 * (k + 1)].T)
             for k in range(KT)], axis=1)          # [128, KT*128]
        bout = b_out[128 * j:128 * (j + 1)][:, None]
        in_maps.append({
            "wsum": wsum, "biasf": biasf,
            "g0": np.ascontiguousarray(g0),
            "wout": wout, "bout": np.ascontiguousarray(bout),
            "ident": ident,
        })
    return in_maps


REP_VARIANT = ""


def kernel_rep(x, W_ih, W_hh, b_ih, b_hh, W_out, b_out, _t_steps=T,
               variant=None):
    if variant is None:
        variant = REP_VARIANT
    key = ("rep", _t_steps, variant)
    if key not in _NC_CACHE:
        _NC_CACHE[key] = _build_rep(_t_steps, variant)
    if key not in _RUNNER_CACHE:
        _RUNNER_CACHE[key] = _CachedSpmdRunner(_NC_CACHE[key])
    in_maps = _prep_inputs_rep(x, W_ih, W_hh, b_ih, b_hh, W_out, b_out,
                               variant)
    results = _RUNNER_CACHE[key].run(in_maps)
    parts = [results[j]["outT"] for j in range(NC)]   # each [128, T, B]
    full = np.concatenate(parts, axis=0)              # [E, T, B]
    return np.ascontiguousarray(full.transpose(2, 1, 0))  # [B, T, E]


def build_for_timing(inputs, t_steps=T, variant=None):
    """(nc, in_maps) for the current kernel() entry point — used by
    work/time_h24.py to time the exact module kernel() runs."""
    if variant is None:
        variant = REP_VARIANT
    key = ("rep", t_steps, variant)
    if key not in _NC_CACHE:
        _NC_CACHE[key] = _build_rep(t_steps, variant)
    return _NC_CACHE[key], _prep_inputs_rep(**inputs, variant=variant)


def kernel(x, W_ih, W_hh, b_ih, b_hh, W_out, b_out, _t_steps=T):
    """Entry point: replicated LSTM + E-sharded projection (no cross-core
    communication; the gates matmul is N-bound so only unit-sharding could
    cut PE time, and every measured exchange mechanism costs more than it
    saves). kernel_h24 is the previous 2x4-hybrid fallback."""
    return kernel_rep(x, W_ih, W_hh, b_ih, b_hh, W_out, b_out, _t_steps=_t_steps)



# revision 7
# speedup vs baseline: 1.1298x; 1.1298x over previous
"""Trainium2 Bass kernel for nn_Decompressor (LSTM decompressor).

Reference computation (see problem statement):
    T=256 steps of an LSTM (batch B=128, hidden P=1024) whose output feeds
    back as its input, followed by a linear projection to E=1024:
        gates_t = xin @ W_ih.T + h @ W_hh.T + (b_ih + b_hh)
        i,f,g,o = split(gates_t); c = sig(f)*c + sig(i)*tanh(g)
        h = sig(o)*tanh(c);  xin_{t+1} = h
        out[:, t, :] = h_t @ W_out.T + b_out
    Since xin == h for t>=1, gates_t = h @ (W_ih + W_hh).T + b for t>=1.

Distribution (8 NeuronCores, SPMD):
    Model-parallel over the hidden dimension: core j owns hidden units
    [128j, 128j+128). Each step, core j computes its 512 gate columns
    (i|f|o|g of its units) as a full-batch matmul (M=B=128, K=1024, N=512,
    float32r), applies the LSTM cell pointwise, transposes its h-slice, and
    an AllGather reassembles the full h^T (the next step's stationary
    operand) on every core. The output projection is sharded over E: core j
    computes out^T[Ej, :, :] from the gathered h^T, batching 2 timesteps per
    matmul (N=256) so float32r runs at full rate.

Host side: weight reordering/slicing, x transpose, and final concat/
transpose of the per-core [E_j, T, B] outputs back to [B, T, E].
"""

import numpy as np
import ml_dtypes

import concourse.bacc as bacc
import concourse.mybir as mybir
import concourse.tile as tile
from concourse.bass_utils import run_bass_kernel_spmd

B = 128
P = 1024
E = 1024
T = 256
NC = 8
KT = P // 128          # 8 K-tiles
GS = 4 * P // NC       # 512 gate columns per core
ES = E // NC           # 128 output columns per core

F32 = mybir.dt.float32
F32R = mybir.dt.float32r


def _build(t_steps=T, variant="full"):
    nc = bacc.Bacc("TRN2", target_bir_lowering=False, debug=False, num_devices=NC)

    wrec_d = nc.dram_tensor("wrec", [128, KT * GS], F32R, kind="ExternalInput").ap()
    win_d = nc.dram_tensor("win", [128, KT * GS], F32R, kind="ExternalInput").ap()
    xT_d = nc.dram_tensor("xT", [128, KT * B], F32R, kind="ExternalInput").ap()
    bias_d = nc.dram_tensor("bias", [1, GS], F32R, kind="ExternalInput").ap()
    biasf_d = nc.dram_tensor("biasf", [128, GS], F32, kind="ExternalInput").ap()
    wout_d = nc.dram_tensor("wout", [128, KT * ES], F32R, kind="ExternalInput").ap()
    bout_d = nc.dram_tensor("bout", [128, 1], F32, kind="ExternalInput").ap()
    ones_d = nc.dram_tensor("ones", [1, 128], F32R, kind="ExternalInput").ap()
    ident_d = nc.dram_tensor("ident", [128, 128], F32R, kind="ExternalInput").ap()
    outT_d = nc.dram_tensor("outT", [ES, t_steps, B], F32, kind="ExternalOutput").ap()

    nb = 4 if "b4" in variant else (3 if "b3" in variant else 2)
    with tile.TileContext(nc) as tc:
        with (
            tc.tile_pool(name="const", bufs=1) as cpool,
            tc.tile_pool(name="state", bufs=1) as spool,
            tc.tile_pool(name="work", bufs=nb) as wk,
            tc.tile_pool(name="pair", bufs=nb) as pairp,
            tc.tile_pool(name="psum", bufs=2, space="PSUM") as psg,
            tc.tile_pool(name="dram", bufs=nb + 1, space="DRAM") as dram,
        ):
            wrec_sb = cpool.tile([128, KT * GS], F32R)
            win_sb = cpool.tile([128, KT * GS], F32R)
            xT_sb = cpool.tile([128, KT * B], F32R)
            bias_sb = cpool.tile([1, GS], F32R)
            biasf_sb = cpool.tile([128, GS], F32)
            wout_sb = cpool.tile([128, KT * ES], F32R)
            bout_sb = cpool.tile([128, 1], F32)
            ones_sb = cpool.tile([1, 128], F32R)
            ident_sb = cpool.tile([128, 128], F32R)
            nc.sync.dma_start(wrec_sb[:], wrec_d[:])
            nc.sync.dma_start(win_sb[:], win_d[:])
            nc.sync.dma_start(xT_sb[:], xT_d[:])
            nc.sync.dma_start(bias_sb[:], bias_d[:])
            nc.sync.dma_start(biasf_sb[:], biasf_d[:])
            nc.sync.dma_start(wout_sb[:], wout_d[:])
            nc.sync.dma_start(bout_sb[:], bout_d[:])
            nc.sync.dma_start(ones_sb[:], ones_d[:])
            nc.sync.dma_start(ident_sb[:], ident_d[:])

            c_sb = spool.tile([128, 128], F32)

            pair = None
            for s in range(t_steps):
                # ---- gates matmul: [B, GS] psum, bias via K=1 matmul ----
                ps_g = psg.tile([128, GS], F32, tag="ps_g")
                nobias = "nobias" in variant or "dvb" in variant
                if not nobias:
                    nc.tensor.matmul(ps_g[:], ones_sb[:], bias_sb[:],
                                     start=True, stop=False)
                w_sb = win_sb if s == 0 else wrec_sb
                for k in range(KT):
                    if s == 0:
                        lhsT = xT_sb[:, k * B:(k + 1) * B]
                    else:
                        slot = (s - 1) % 2
                        lhsT = pair[:, slot * P + k * 128: slot * P + (k + 1) * 128]
                    nc.tensor.matmul(ps_g[:], lhsT, w_sb[:, k * GS:(k + 1) * GS],
                                     start=(nobias and k == 0), stop=(k == KT - 1))

                # ---- pointwise: layout [i|f|o|g] each 128 wide ----
                gates_sb = wk.tile([128, GS], F32, tag="gates")
                if "dvb" in variant:
                    gpre = wk.tile([128, GS], F32, tag="gpre")
                    nc.vector.tensor_tensor(gpre[:], ps_g[:], biasf_sb[:],
                                            mybir.AluOpType.add)
                    act_src = gpre
                else:
                    act_src = ps_g
                nc.scalar.activation(gates_sb[:, 0:384], act_src[:, 0:384],
                                     mybir.ActivationFunctionType.Sigmoid)
                nc.scalar.activation(gates_sb[:, 384:512], act_src[:, 384:512],
                                     mybir.ActivationFunctionType.Tanh)
                i_ap = gates_sb[:, 0:128]
                f_ap = gates_sb[:, 128:256]
                o_ap = gates_sb[:, 256:384]
                g_ap = gates_sb[:, 384:512]
                if s == 0:
                    nc.vector.tensor_tensor(c_sb[:], i_ap, g_ap,
                                            mybir.AluOpType.mult)
                else:
                    ig = wk.tile([128, 128], F32, tag="ig")
                    nc.vector.tensor_tensor(ig[:], i_ap, g_ap,
                                            mybir.AluOpType.mult)
                    nc.vector.tensor_tensor(c_sb[:], c_sb[:], f_ap,
                                            mybir.AluOpType.mult)
                    nc.vector.tensor_tensor(c_sb[:], c_sb[:], ig[:],
                                            mybir.AluOpType.add)
                th = wk.tile([128, 128], F32, tag="th")
                nc.scalar.activation(th[:], c_sb[:],
                                     mybir.ActivationFunctionType.Tanh)
                h_sb = wk.tile([128, 128], F32R, tag="h")
                nc.vector.tensor_tensor(h_sb[:], o_ap, th[:],
                                        mybir.AluOpType.mult)

                # ---- transpose h slice -> [units, B] and exchange ----
                ps_t = psg.tile([128, 128], F32R, tag="ps_t")
                nc.tensor.transpose(ps_t[:], h_sb[:], ident_sb[:])
                hT_send = wk.tile([128, 128], F32R, tag="hTs")
                if "acp" in variant:
                    nc.scalar.copy(hT_send[:], ps_t[:])
                else:
                    nc.vector.tensor_copy(hT_send[:], ps_t[:])

                if "sbex" in variant:
                    if s % 2 == 0:
                        pair = pairp.tile([128, 2 * P], F32R, tag="pair")
                    slot = s % 2
                    for k in range(KT):
                        nc.vector.tensor_copy(
                            pair[:, slot * P + k * 128: slot * P + (k + 1) * 128],
                            hT_send[:])
                else:
                    inb = dram.tile([128, B], F32R, tag="inb")
                    outb = dram.tile([NC * 128, B], F32R, tag="outb")
                    (nc.scalar if "sdma" in variant else nc.sync).dma_start(inb[:], hT_send[:])
                if "sbex" in variant:
                    pass
                elif "noag" not in variant:
                    nc.gpsimd.collective_compute(
                        "AllGather", mybir.AluOpType.bypass,
                        ins=[inb.opt()], outs=[outb.opt()],
                        replica_groups=[list(range(NC))],
                    )
                if "sbex" in variant:
                    pass
                elif True:
                    if s % 2 == 0:
                        pair = pairp.tile([128, 2 * P], F32R, tag="pair")
                slot = s % 2
                if "sbex" in variant:
                    pass
                elif "noag" in variant:
                    for k in range(KT):
                        nc.sync.dma_start(
                            pair[:, slot * P + k * 128: slot * P + (k + 1) * 128],
                            inb.opt(),
                        )
                else:
                    src3 = outb.opt().rearrange("(k p) b -> p k b", p=128)
                    for k in range(KT):
                        eng = nc.scalar if ("sdma" in variant and k % 2 == 1) else nc.sync
                        eng.dma_start(
                            pair[:, slot * P + k * 128: slot * P + (k + 1) * 128],
                            src3[:, k, :],
                        )

                # ---- projection of steps (s-1, s) every odd step ----
                if "noproj" in variant:
                    continue
                if s % 2 == 1 or s == t_steps - 1:
                    if s % 2 == 1:
                        nsteps, base = 2, s - 1
                    else:  # odd t_steps tail: project the single last step
                        nsteps, base = 1, s
                    ps_p = psg.tile([128, nsteps * B], F32, tag="ps_p")
                    rhs4 = pair[:].rearrange("p (s2 k b) -> p s2 k b", s2=2, k=KT)
                    for k in range(KT):
                        rhs = rhs4[:, 0:nsteps, k, :]
                        nc.tensor.matmul(ps_p[:], wout_sb[:, k * ES:(k + 1) * ES],
                                         rhs, start=(k == 0), stop=(k == KT - 1))
                    out_sb = wk.tile([128, nsteps * B], F32, tag="out_sb")
                    nc.scalar.activation(out_sb[:], ps_p[:],
                                         mybir.ActivationFunctionType.Identity,
                                         bias=bout_sb[:, 0:1])
                    dst = outT_d[:, base:base + nsteps, :]
                    nc.sync.dma_start(
                        dst, out_sb[:].rearrange("p (s2 b) -> p s2 b", s2=nsteps))

    nc.compile()
    return nc


def _prep_inputs(x, W_ih, W_hh, b_ih, b_hh, W_out, b_out):
    x = np.asarray(x, np.float32)
    W_ih = np.asarray(W_ih, np.float32)
    W_hh = np.asarray(W_hh, np.float32)
    b_ih = np.asarray(b_ih, np.float32)
    b_hh = np.asarray(b_hh, np.float32)
    W_out = np.asarray(W_out, np.float32)
    b_out = np.asarray(b_out, np.float32)

    Wsum = W_ih + W_hh
    bsum = b_ih + b_hh
    # xT as [8 K-tiles, 128, B] laid out [128, KT*B] per tile columns:
    xT_tiles = np.ascontiguousarray(x.T).reshape(KT, 128, B)
    xT_flat = np.concatenate([xT_tiles[k] for k in range(KT)], axis=1)

    ones = np.ones((1, 128), np.float32)
    ident = np.eye(128, dtype=np.float32)

    in_maps = []
    for j in range(NC):
        js = np.arange(128 * j, 128 * (j + 1))
        rows = np.concatenate([0 * P + js, 1 * P + js, 3 * P + js, 2 * P + js])
        Wj = Wsum[rows, :]                       # [512, 1024]
        wrec_t = np.ascontiguousarray(Wj.T).reshape(KT, 128, GS)
        wrec = np.concatenate([wrec_t[k] for k in range(KT)], axis=1)
        Wji = W_ih[rows, :]
        win_t = np.ascontiguousarray(Wji.T).reshape(KT, 128, GS)
        win = np.concatenate([win_t[k] for k in range(KT)], axis=1)
        bias = bsum[rows][None, :]
        Woj = W_out[128 * j:128 * (j + 1), :]    # [128, 1024]
        wout_t = np.ascontiguousarray(Woj.T).reshape(KT, 128, ES)
        wout = np.concatenate([wout_t[k] for k in range(KT)], axis=1)
        bout = b_out[128 * j:128 * (j + 1)][:, None]
        in_maps.append({
            "biasf": np.broadcast_to(bias, (128, GS)).copy(),
            "wrec": np.ascontiguousarray(wrec),
            "win": np.ascontiguousarray(win),
            "xT": np.ascontiguousarray(xT_flat),
            "bias": np.ascontiguousarray(bias),
            "wout": np.ascontiguousarray(wout),
            "bout": np.ascontiguousarray(bout),
            "ones": ones,
            "ident": ident,
        })
    return in_maps


_NC_CACHE = {}


def kernel_mp8(x, W_ih, W_hh, b_ih, b_hh, W_out, b_out, _t_steps=T):
    if _t_steps not in _NC_CACHE:
        _NC_CACHE[_t_steps] = _build(_t_steps)
    nc = _NC_CACHE[_t_steps]
    in_maps = _prep_inputs(x, W_ih, W_hh, b_ih, b_hh, W_out, b_out)
    res = run_bass_kernel_spmd(nc, in_maps, list(range(NC)))
    parts = [res.results[j]["outT"] for j in range(NC)]   # each [ES, T, B]
    full = np.concatenate(parts, axis=0)                  # [E, T, B]
    return np.ascontiguousarray(full.transpose(2, 1, 0))  # [B, T, E]


# ---------------- 2x4 hybrid: batch halves x 4-way unit shard ----------------
# cores {0-3}: batch rows 0:64, cores {4-7}: rows 64:128 (replica groups
# [[0,1,2,3],[4,5,6,7]] run their AllGathers CONCURRENTLY). Each core owns 256
# hidden units as two 128-unit blocks (a/b); per K-tile one LDW serves both
# blocks' matmuls. Projection: 256 E-cols per core, batched over 4 steps
# (N=4*64=256) from a "quad" buffer.
BL = 64           # local batch
UB = 2            # unit blocks per core


def _build_h24(t_steps=T):
    nc = bacc.Bacc("TRN2", target_bir_lowering=False, debug=False, num_devices=NC)
    GS2 = 1024    # gate cols per core (2 blocks x 512)
    ES2 = 256     # E cols per core

    wrec_d = nc.dram_tensor("wrec", [128, KT * GS2], F32R, kind="ExternalInput").ap()
    win_d = nc.dram_tensor("win", [128, KT * GS2], F32R, kind="ExternalInput").ap()
    xT_d = nc.dram_tensor("xT", [128, KT * BL], F32R, kind="ExternalInput").ap()
    bias_d = nc.dram_tensor("bias", [1, GS2], F32R, kind="ExternalInput").ap()
    wout_d = nc.dram_tensor("wout", [128, KT * ES2], F32R, kind="ExternalInput").ap()
    bout_d = nc.dram_tensor("bout", [128, UB], F32, kind="ExternalInput").ap()
    ones_d = nc.dram_tensor("ones", [1, BL], F32R, kind="ExternalInput").ap()
    ident_d = nc.dram_tensor("ident", [128, 128], F32R, kind="ExternalInput").ap()
    outT_d = nc.dram_tensor("outT", [ES2, t_steps, BL], F32, kind="ExternalOutput").ap()

    with tile.TileContext(nc) as tc:
        with (
            tc.tile_pool(name="const", bufs=1) as cpool,
            tc.tile_pool(name="state", bufs=1) as spool,
            tc.tile_pool(name="work", bufs=2) as wk,
            tc.tile_pool(name="quad", bufs=2) as quadp,
            tc.tile_pool(name="psum", bufs=2, space="PSUM") as psg,
            tc.tile_pool(name="dram", bufs=3, space="DRAM") as dram,
        ):
            wrec_sb = cpool.tile([128, KT * GS2], F32R)
            win_sb = cpool.tile([128, KT * GS2], F32R)
            xT_sb = cpool.tile([128, KT * BL], F32R)
            bias_sb = cpool.tile([1, GS2], F32R)
            wout_sb = cpool.tile([128, KT * ES2], F32R)
            bout_sb = cpool.tile([128, UB], F32)
            ones_sb = cpool.tile([1, BL], F32R)
            ident_sb = cpool.tile([128, 128], F32R)
            nc.sync.dma_start(wrec_sb[:], wrec_d[:])
            nc.sync.dma_start(win_sb[:], win_d[:])
            nc.sync.dma_start(xT_sb[:], xT_d[:])
            nc.sync.dma_start(bias_sb[:], bias_d[:])
            nc.sync.dma_start(wout_sb[:], wout_d[:])
            nc.sync.dma_start(bout_sb[:], bout_d[:])
            nc.sync.dma_start(ones_sb[:], ones_d[:])
            nc.sync.dma_start(ident_sb[:], ident_d[:])

            c_sb = spool.tile([128, 2 * 128], F32)   # [64used, block v cols]

            quad = None
            SLOTW = KT * BL            # 512 cols per step-slot in quad
            for s in range(t_steps):
                # gates for both unit blocks; one K-tile LDW serves both MMs
                pss = []
                for v in range(UB):
                    ps_v = psg.tile([128, GS], F32, tag=f"ps_g{v}")
                    nc.tensor.matmul(ps_v[0:BL, :], ones_sb[:],
                                     bias_sb[:, v * GS:(v + 1) * GS],
                                     start=True, stop=False)
                    pss.append(ps_v)
                w_sb = win_sb if s == 0 else wrec_sb
                for k in range(KT):
                    if s == 0:
                        lhsT = xT_sb[:, k * BL:(k + 1) * BL]
                    else:
                        slot = (s - 1) % 4
                        lhsT = quad_prev[:, slot * SLOTW + k * BL: slot * SLOTW + (k + 1) * BL]
                    for v in range(UB):
                        nc.tensor.matmul(
                            pss[v][0:BL, :], lhsT,
                            w_sb[:, (k * UB + v) * GS:(k * UB + v + 1) * GS],
                            start=False, stop=(k == KT - 1))

                ps_t = psg.tile([128, 128], F32R, tag="ps_t")
                for v in range(UB):
                    gates_sb = wk.tile([BL, GS], F32, tag=f"gates{v}")
                    nc.scalar.activation(gates_sb[:, 0:384], pss[v][0:BL, 0:384],
                                         mybir.ActivationFunctionType.Sigmoid)
                    nc.scalar.activation(gates_sb[:, 384:512], pss[v][0:BL, 384:512],
                                         mybir.ActivationFunctionType.Tanh)
                    i_ap = gates_sb[:, 0:128]
                    f_ap = gates_sb[:, 128:256]
                    o_ap = gates_sb[:, 256:384]
                    g_ap = gates_sb[:, 384:512]
                    c_v = c_sb[0:BL, v * 128:(v + 1) * 128]
                    if s == 0:
                        nc.vector.tensor_tensor(c_v, i_ap, g_ap, mybir.AluOpType.mult)
                    else:
                        ig = wk.tile([BL, 128], F32, tag=f"ig{v}")
                        nc.vector.tensor_tensor(ig[:], i_ap, g_ap, mybir.AluOpType.mult)
                        nc.vector.tensor_tensor(c_v, c_v, f_ap, mybir.AluOpType.mult)
                        nc.vector.tensor_tensor(c_v, c_v, ig[:], mybir.AluOpType.add)
                    th = wk.tile([BL, 128], F32, tag=f"th{v}")
                    nc.scalar.activation(th[:], c_v, mybir.ActivationFunctionType.Tanh)
                    h_sb = wk.tile([BL, 128], F32R, tag=f"h{v}")
                    nc.vector.tensor_tensor(h_sb[:], o_ap, th[:], mybir.AluOpType.mult)
                    # transpose block v: [64,128] -> [128,64] at ps_t cols v*64
                    nc.tensor.transpose(ps_t[:, v * BL:(v + 1) * BL], h_sb[:],
                                        ident_sb[0:BL, 0:BL])
                hT_send = wk.tile([128, UB * BL], F32R, tag="hTs")
                nc.vector.tensor_copy(hT_send[:], ps_t[:])

                inb = dram.tile([UB * 128, BL], F32R, tag="inb")
                outb = dram.tile([NC // 2 * UB * 128, BL], F32R, tag="outb")
                nc.sync.dma_start(
                    inb.opt().rearrange("(v p) b -> p v b", p=128),
                    hT_send[:].rearrange("p (v b) -> p v b", v=UB))
                nc.gpsimd.collective_compute(
                    "AllGather", mybir.AluOpType.bypass,
                    ins=[inb.opt()], outs=[outb.opt()],
                    replica_groups=[[0, 1, 2, 3], [4, 5, 6, 7]],
                )
                if s % 4 == 0:
                    quad = quadp.tile([128, 4 * SLOTW], F32R, tag="quad")
                slot = s % 4
                src3 = outb.opt().rearrange("(k p) b -> p k b", p=128)
                for k in range(KT):
                    nc.sync.dma_start(
                        quad[:, slot * SLOTW + k * BL: slot * SLOTW + (k + 1) * BL],
                        src3[:, k, :])
                quad_prev = quad

                # projection of steps (s-3..s) every 4th step
                if s % 4 == 3 or s == t_steps - 1:
                    nst = 4 if s % 4 == 3 else (s % 4) + 1
                    base = s - nst + 1
                    rhs5 = quad[:].rearrange("p (s4 k b) -> p s4 k b", s4=4, k=KT)
                    for m in range(UB):
                        ps_p = psg.tile([128, nst * BL], F32, tag="ps_p")
                        for k in range(KT):
                            nc.tensor.matmul(
                                ps_p[:], wout_sb[:, (k * UB + m) * 128:(k * UB + m + 1) * 128],
                                rhs5[:, 0:nst, k, :],
                                start=(k == 0), stop=(k == KT - 1))
                        out_sb = wk.tile([128, nst * BL], F32, tag="out_sb")
                        nc.scalar.activation(out_sb[:], ps_p[:],
                                             mybir.ActivationFunctionType.Identity,
                                             bias=bout_sb[:, m:m + 1])
                        nc.sync.dma_start(
                            outT_d[m * 128:(m + 1) * 128, base:base + nst, :],
                            out_sb[:].rearrange("p (s4 b) -> p s4 b", s4=nst))

    nc.compile()
    return nc


def _prep_inputs_h24(x, W_ih, W_hh, b_ih, b_hh, W_out, b_out):
    x = np.asarray(x, np.float32); W_ih = np.asarray(W_ih, np.float32)
    W_hh = np.asarray(W_hh, np.float32); b_ih = np.asarray(b_ih, np.float32)
    b_hh = np.asarray(b_hh, np.float32); W_out = np.asarray(W_out, np.float32)
    b_out = np.asarray(b_out, np.float32)
    Wsum = W_ih + W_hh
    bsum = b_ih + b_hh
    ones = np.ones((1, BL), np.float32)
    ident = np.eye(128, dtype=np.float32)
    GS2, ES2 = 1024, 256
    in_maps = []
    for j in range(NC):
        half, u = j // 4, j % 4
        xh = x[64 * half:64 * (half + 1), :]                     # [64, 1024]
        xT_t = np.ascontiguousarray(xh.T).reshape(KT, 128, BL)
        xT = np.concatenate([xT_t[k] for k in range(KT)], axis=1)
        # unit blocks a/b: units [256u, 256u+128), [256u+128, 256u+256)
        rows = []
        for v in range(UB):
            js = np.arange(256 * u + 128 * v, 256 * u + 128 * (v + 1))
            rows.append(np.concatenate([0 * P + js, 1 * P + js, 3 * P + js, 2 * P + js]))

        def ktiles(Wmat):
            # per K-tile k: [block_a 512 | block_b 512]
            cols = []
            for k in range(KT):
                seg = [np.ascontiguousarray(Wmat[r, 128 * k:128 * (k + 1)].T)
                       for r in rows]          # each [128, 512]
                cols.append(np.concatenate(seg, axis=1))
            return np.concatenate(cols, axis=1)

        wrec = ktiles(Wsum)
        win = ktiles(W_ih)
        bias = np.concatenate([bsum[r] for r in rows])[None, :]
        # wout: E block = [256j%4... E cols 256u2 where u2 = u (any fixed map)
        ecols = np.arange(256 * u, 256 * (u + 1))
        wo = []
        for k in range(KT):
            seg = [np.ascontiguousarray(
                W_out[ecols[128 * m:128 * (m + 1)], 128 * k:128 * (k + 1)].T)
                for m in range(UB)]           # each [128, 128]
            wo.append(np.concatenate(seg, axis=1))
        wout = np.concatenate(wo, axis=1)
        bout = np.stack([b_out[ecols[128 * m:128 * (m + 1)]] for m in range(UB)], axis=1)
        in_maps.append({
            "wrec": np.ascontiguousarray(wrec), "win": np.ascontiguousarray(win),
            "xT": np.ascontiguousarray(xT), "bias": np.ascontiguousarray(bias),
            "wout": np.ascontiguousarray(wout), "bout": np.ascontiguousarray(bout),
            "ones": ones, "ident": ident,
        })
    return in_maps




# One PJRT executable per built module, reused across kernel() calls.
# run_bass_kernel_spmd builds a fresh executable every call; alternating
# executables that contain collectives within one process can desync the
# mesh (observed), so execute through a single cached callable instead.
_RUNNER_CACHE = {}


class _CachedSpmdRunner:
    def __init__(self, nc):
        import jax
        from jax.sharding import Mesh, PartitionSpec
        from jax.experimental.shard_map import shard_map
        from concourse.bass2jax import (
            _bass_exec_p, install_neuronx_cc_hook, partition_id_tensor)

        install_neuronx_cc_hook()
        partition_name = (
            nc.partition_id_tensor.name if nc.partition_id_tensor else None)
        in_names, out_names, out_avals, zero_outs = [], [], [], []
        for alloc in nc.m.functions[0].allocations:
            if not isinstance(alloc, mybir.MemoryLocationSet):
                continue
            name = alloc.memorylocations[0].name
            if alloc.kind == "ExternalInput":
                if name != partition_name:
                    in_names.append(name)
            elif alloc.kind == "ExternalOutput":
                out_names.append(name)
                shape = tuple(alloc.tensor_shape)
                dtype = mybir.dt.np(alloc.dtype)
                out_avals.append(jax.core.ShapedArray(shape, dtype))
                zero_outs.append(np.zeros(shape, dtype))
        n_params, n_outs = len(in_names), len(out_avals)
        all_in = list(in_names) + list(out_names)
        if partition_name is not None:
            all_in.append(partition_name)

        def _body(*args):
            operands = list(args)
            if partition_name is not None:
                operands.append(partition_id_tensor())
            return tuple(_bass_exec_p.bind(
                *operands, out_avals=tuple(out_avals),
                in_names=tuple(all_in), out_names=tuple(out_names),
                lowering_input_output_aliases=(),
                sim_require_finite=True, sim_require_nnan=True, nc=nc))

        devices = jax.devices()[:NC]
        mesh = Mesh(np.asarray(devices), ("core",))
        self._fn = jax.jit(
            shard_map(_body, mesh=mesh,
                      in_specs=(PartitionSpec("core"),) * (n_params + n_outs),
                      out_specs=(PartitionSpec("core"),) * n_outs,
                      check_rep=False),
            keep_unused=True)
        self._jax = jax
        self.in_names, self.out_names, self.out_avals = in_names, out_names, out_avals
        self._zeros = [
            np.zeros((NC * z.shape[0], *z.shape[1:]), z.dtype) for z in zero_outs]

    def run(self, in_maps):
        per_core = [[np.asarray(m[n]) for n in self.in_names] for m in in_maps]
        concat_in = [
            np.concatenate([per_core[c][i] for c in range(NC)], axis=0)
            for i in range(len(self.in_names))]
        out_arrs = self._fn(*concat_in, *self._zeros)
        self._jax.block_until_ready(out_arrs)
        return [
            {n: np.asarray(out_arrs[i]).reshape(NC, *self.out_avals[i].shape)[c]
             for i, n in enumerate(self.out_names)}
            for c in range(NC)]


def kernel_h24(x, W_ih, W_hh, b_ih, b_hh, W_out, b_out, _t_steps=T):
    key = ("h24", _t_steps)
    if key not in _NC_CACHE:
        _NC_CACHE[key] = _build_h24(_t_steps)
    if key not in _RUNNER_CACHE:
        _RUNNER_CACHE[key] = _CachedSpmdRunner(_NC_CACHE[key])
    in_maps = _prep_inputs_h24(x, W_ih, W_hh, b_ih, b_hh, W_out, b_out)
    results = _RUNNER_CACHE[key].run(in_maps)
    out = np.zeros((128, _t_steps, 1024), np.float32)
    for j in range(NC):
        half, u = j // 4, j % 4
        part = results[j]["outT"]            # [256 E, T, 64 B]
        out[64 * half:64 * (half + 1), :, 256 * u:256 * (u + 1)] = part.transpose(2, 1, 0)
    return out


# ---------------- replicated LSTM, E-sharded projection ----------------
# Every core runs the full recurrence (no cross-core traffic at all): the
# gates matmul is N-bound on the PE (64 matmuls of [128K,128]x[128,512]
# fp32r at 1 cycle/row), so sharding batch would not reduce per-step time,
# and per-step collectives measured 12-54us -- far above the 13.6us PE
# floor. Only the output projection is sharded (core j owns E columns
# [128j, 128j+128)), batched 2 timesteps per matmul (N=256, full fp32r
# rate). Step 0's pre-activation gates (x @ W_ih.T + b) come from the
# host, so W_ih/x never touch the device. Bias is added by the Pool
# engine (PSUM + broadcast bias -> SBUF), keeping the PE at its floor.
# h^T lives in a 4-parity ring so the 2-step projection window is a
# contiguous slice; gate columns are host-permuted to [i|f|o|g] per
# 128-unit block so each 512-col PSUM chunk is one complete block.

F8 = mybir.dt.float8e4
DRPM = mybir.MatmulPerfMode.DoubleRow
S_H8 = 16.0        # dr8: h scale into fp8e4m3
S_W8 = 2048.0      # dr8: recurrent weight scale into fp8e4m3 (max 240!)
INV8 = 1.0 / (S_H8 * S_W8)


def _build_rep(t_steps=T, variant=""):
    nc = bacc.Bacc("TRN2", target_bir_lowering=False, debug=False,
                   num_devices=NC)
    dr8 = "dr8" in variant
    GALL = 4 * P                    # 4096 gate cols
    if dr8:
        # fp8 DoubleRow weights: per (chunk ch, kd) a [128, 2, 512] block
        w8_d = nc.dram_tensor("w8", [128, 32 * 1024], F8,
                              kind="ExternalInput").ap()
    else:
        wsum_d = nc.dram_tensor("wsum", [128, KT * GALL], F32R,
                                kind="ExternalInput").ap()
    biasf_d = nc.dram_tensor("biasf", [128, GALL], F32,
                             kind="ExternalInput").ap()
    g0_d = nc.dram_tensor("g0", [128, GALL], F32, kind="ExternalInput").ap()
    wout_d = nc.dram_tensor("wout", [128, KT * 128], F32R,
                            kind="ExternalInput").ap()
    bout_d = nc.dram_tensor("bout", [128, 1], F32, kind="ExternalInput").ap()
    ident_d = nc.dram_tensor("ident", [128, 128], F32R,
                             kind="ExternalInput").ap()
    outT_d = nc.dram_tensor("outT", [128, t_steps, B], F32,
                            kind="ExternalOutput").ap()

    CW = 1024 if "mm1024" in variant else 512   # psum chunk width
    NCH = GALL // CW                            # chunks per step
    Q = CW // 4                                 # units per chunk
    NSUB = Q // 128                             # 128-unit sub-blocks
    with tile.TileContext(nc) as tc:
        with (
            tc.tile_pool(name="const", bufs=1) as cpool,
            tc.tile_pool(name="state", bufs=1) as spool,
            tc.tile_pool(name="work", bufs=3) as wk,
            tc.tile_pool(name="ht8", bufs=2) as hp,
            tc.tile_pool(name="psg", bufs=(2 if CW == 1024 else 3),
                         space="PSUM") as psg,
            tc.tile_pool(name="pst", bufs=2, space="PSUM") as pst,
            tc.tile_pool(name="psp", bufs=2, space="PSUM") as psp,
        ):
            if dr8:
                w8_sb = cpool.tile([128, 32 * 1024], F8)
            else:
                wsum_sb = cpool.tile([128, KT * GALL], F32R)
            biasf_sb = cpool.tile([128, GALL], F32)
            wout_sb = cpool.tile([128, KT * 128], F32R)
            bout_sb = cpool.tile([128, 1], F32)
            ident_sb = cpool.tile([128, 128], F32R)
            if dr8:
                nc.sync.dma_start(w8_sb[:], w8_d[:])
            else:
                nc.sync.dma_start(wsum_sb[:], wsum_d[:])
            nc.sync.dma_start(biasf_sb[:], biasf_d[:])
            nc.sync.dma_start(wout_sb[:], wout_d[:])
            nc.sync.dma_start(bout_sb[:], bout_d[:])
            nc.sync.dma_start(ident_sb[:], ident_d[:])

            hT = spool.tile([128, 4 * P], F32R)      # 4 step-parities
            c_sb = spool.tile([128, P], F32)
            hT8_prev = None

            for s in range(t_steps):
                par = s % 4
                prev = (s - 1) % 4
                if dr8:
                    hT8 = hp.tile([128, 1024], F8, tag="hT8")
                for ch in range(NCH):
                    if s == 0:
                        pre = wk.tile([128, CW], F32, tag="pre")
                        nc.sync.dma_start(pre[:],
                                          g0_d[:, ch * CW:(ch + 1) * CW])
                    else:
                        ps_g = psg.tile([128, CW], F32, tag="ps_g")
                        if dr8:
                            for kd in range(4):
                                lhsT = hT8_prev[:, kd * 256:(kd + 1) * 256]
                                lhsT = lhsT.rearrange(
                                    "p (two m) -> p two m", two=2)
                                woff = (ch * 4 + kd) * 1024
                                rhs = w8_sb[:, woff:woff + 1024].rearrange(
                                    "p (two n) -> p two n", two=2)
                                nc.tensor.matmul(
                                    ps_g[:], lhsT, rhs, start=(kd == 0),
                                    stop=(kd == 3), perf_mode=DRPM)
                        else:
                            for k in range(KT):
                                nc.tensor.matmul(
                                    ps_g[:],
                                    hT[:, k * 512 + prev * 128:
                                       k * 512 + (prev + 1) * 128],
                                    wsum_sb[:, (k * NCH + ch) * CW:
                                            (k * NCH + ch + 1) * CW],
                                    start=(k == 0), stop=(k == KT - 1))
                        pre = wk.tile([128, CW], F32, tag="pre")
                        nc.vector.tensor_tensor(
                            pre[:], ps_g[:],
                            biasf_sb[:, ch * CW:(ch + 1) * CW],
                            mybir.AluOpType.add)
                    gates = wk.tile([128, CW], F32, tag="gates")
                    sc8 = INV8 if (dr8 and s > 0) else 1.0
                    nc.scalar.activation(gates[:, 0:3 * Q], pre[:, 0:3 * Q],
                                         mybir.ActivationFunctionType.Sigmoid,
                                         scale=sc8)
                    nc.scalar.activation(gates[:, 3 * Q:CW], pre[:, 3 * Q:CW],
                                         mybir.ActivationFunctionType.Tanh,
                                         scale=sc8)
                    for w in range(NSUB):
                        i_ap = gates[:, w * 128:(w + 1) * 128]
                        f_ap = gates[:, Q + w * 128:Q + (w + 1) * 128]
                        o_ap = gates[:, 2 * Q + w * 128:2 * Q + (w + 1) * 128]
                        g_ap = gates[:, 3 * Q + w * 128:3 * Q + (w + 1) * 128]
                        u = ch * NSUB + w
                        cu = c_sb[:, u * 128:(u + 1) * 128]
                        if s == 0:
                            nc.vector.tensor_tensor(cu, i_ap, g_ap,
                                                    mybir.AluOpType.mult)
                        else:
                            ig = wk.tile([128, 128], F32, tag="ig")
                            nc.gpsimd.tensor_tensor(ig[:], i_ap, g_ap,
                                                    mybir.AluOpType.mult)
                            nc.vector.tensor_tensor(cu, cu, f_ap,
                                                    mybir.AluOpType.mult)
                            nc.vector.tensor_tensor(cu, cu, ig[:],
                                                    mybir.AluOpType.add)
                        th = wk.tile([128, 128], F32, tag="th")
                        nc.scalar.activation(th[:], cu,
                                             mybir.ActivationFunctionType.Tanh)
                        h_u = wk.tile([128, 128], F32R, tag="h")
                        nc.vector.tensor_tensor(h_u[:], o_ap, th[:],
                                                mybir.AluOpType.mult)
                        if "notrans" not in variant:
                            ps_t = pst.tile([128, 128], F32R, tag="ps_t")
                            nc.tensor.transpose(ps_t[:], h_u[:], ident_sb[:])
                            nc.scalar.copy(
                                hT[:, u * 512 + par * 128:
                                   u * 512 + (par + 1) * 128],
                                ps_t[:])
                            if dr8 and u % 2 == 1:
                                pp = u // 2
                                src8 = hT[:].rearrange(
                                    "p (u8 par4 b) -> p u8 par4 b",
                                    u8=8, par4=4, b=128)[:, u - 1:u + 1, par, :]
                                nc.vector.tensor_scalar_mul(
                                    hT8[:, pp * 256:(pp + 1) * 256].rearrange(
                                        "p (two b) -> p two b", two=2),
                                    src8, S_H8)

                if dr8:
                    hT8_prev = hT8

                if "noproj" in variant:
                    continue
                if s % 4 == 3 or s == t_steps - 1:
                    nsteps = 4 if s % 4 == 3 else (s % 4) + 1
                    base = s - nsteps + 1        # base % 4 == 0
                    # k-major hT layout: parities base..s of k-tile k sit at
                    # [k*512, k*512 + nsteps*128) — one contiguous rhs.
                    ps_p = psp.tile([128, nsteps * B], F32, tag="ps_p")
                    for k in range(KT):
                        nc.tensor.matmul(
                            ps_p[:], wout_sb[:, k * 128:(k + 1) * 128],
                            hT[:, k * 512:k * 512 + nsteps * 128],
                            start=(k == 0), stop=(k == KT - 1))
                    out_sb = wk.tile([128, nsteps * B], F32, tag="out_sb")
                    nc.scalar.activation(out_sb[:], ps_p[:],
                                         mybir.ActivationFunctionType.Identity,
                                         bias=bout_sb[:, 0:1])
                    nc.sync.dma_start(
                        outT_d[:, base:s + 1, :],
                        out_sb[:].rearrange("p (st b) -> p st b", st=nsteps))

    nc.compile()
    return nc


def _prep_inputs_rep(x, W_ih, W_hh, b_ih, b_hh, W_out, b_out, variant=""):
    dr8 = "dr8" in variant
    x = np.asarray(x, np.float32)
    W_ih = np.asarray(W_ih, np.float32)
    W_hh = np.asarray(W_hh, np.float32)
    b_ih = np.asarray(b_ih, np.float32)
    b_hh = np.asarray(b_hh, np.float32)
    W_out = np.asarray(W_out, np.float32)
    b_out = np.asarray(b_out, np.float32)

    Wsum = W_ih + W_hh
    bsum = b_ih + b_hh
    # gate-col permutation: per Q-unit chunk ch, [i_ch | f_ch | o_ch | g_ch]
    # (torch row order is i,f,g,o -> indices 0,1,3,2)
    Q = 256 if "mm1024" in variant else 128
    allrows = np.concatenate([
        np.concatenate([q * P + np.arange(Q * u, Q * (u + 1))
                        for q in (0, 1, 3, 2)])
        for u in range(P // Q)])
    Wperm = Wsum[allrows, :]                       # [4096, 1024]
    if dr8:
        Wq = (Wperm * S_W8).astype(ml_dtypes.float8_e4m3)
        assert np.isfinite(Wq.astype(np.float32)).all()
        w8 = np.zeros((128, 32 * 1024), ml_dtypes.float8_e4m3)
        for ch in range(8):
            for kd in range(4):
                base = (ch * 4 + kd) * 1024
                for pl in range(2):
                    blk = Wq[ch * 512:(ch + 1) * 512,
                             kd * 256 + pl * 128:kd * 256 + (pl + 1) * 128]
                    w8[:, base + pl * 512:base + (pl + 1) * 512] = blk.T
        biasf = np.broadcast_to(bsum[allrows] * (S_H8 * S_W8),
                                (128, 4 * P)).copy().astype(np.float32)
    else:
        wsum = np.concatenate(
            [np.ascontiguousarray(Wperm[:, 128 * k:128 * (k + 1)].T)
             for k in range(KT)], axis=1)              # [128, KT*4096]
        biasf = np.broadcast_to(bsum[allrows], (128, 4 * P)).copy()
    g0 = (x @ W_ih.T + bsum)[:, allrows]           # [128, 4096]
    ident = np.eye(128, dtype=np.float32)

    in_maps = []
    for j in range(NC):
        wout = np.concatenate(
            [np.ascontiguousarray(
                W_out[128 * j:128 * (j + 1), 128 * k:128 * (k + 1)].T)
             for k in range(KT)], axis=1)          # [128, KT*128]
        bout = b_out[128 * j:128 * (j + 1)][:, None]
        m = {
            "biasf": biasf,
            "g0": np.ascontiguousarray(g0),
            "wout": wout, "bout": np.ascontiguousarray(bout),
            "ident": ident,
        }
        if dr8:
            m["w8"] = w8
        else:
            m["wsum"] = wsum
        in_maps.append(m)
    return in_maps


REP_VARIANT = ""


def kernel_rep(x, W_ih, W_hh, b_ih, b_hh, W_out, b_out, _t_steps=T,
               variant=None):
    if variant is None:
        variant = REP_VARIANT
    key = ("rep", _t_steps, variant)
    if key not in _NC_CACHE:
        _NC_CACHE[key] = _build_rep(_t_steps, variant)
    if key not in _RUNNER_CACHE:
        _RUNNER_CACHE[key] = _CachedSpmdRunner(_NC_CACHE[key])
    in_maps = _prep_inputs_rep(x, W_ih, W_hh, b_ih, b_hh, W_out, b_out,
                               variant)
    results = _RUNNER_CACHE[key].run(in_maps)
    parts = [results[j]["outT"] for j in range(NC)]   # each [128, T, B]
    full = np.concatenate(parts, axis=0)              # [E, T, B]
    return np.ascontiguousarray(full.transpose(2, 1, 0))  # [B, T, E]


def build_for_timing(inputs, t_steps=T, variant=None):
    """(nc, in_maps) for the current kernel() entry point — used by
    work/time_h24.py to time the exact module kernel() runs."""
    if variant is None:
        variant = REP_VARIANT
    key = ("rep", t_steps, variant)
    if key not in _NC_CACHE:
        _NC_CACHE[key] = _build_rep(t_steps, variant)
    return _NC_CACHE[key], _prep_inputs_rep(**inputs, variant=variant)


def kernel(x, W_ih, W_hh, b_ih, b_hh, W_out, b_out, _t_steps=T):
    """Entry point: replicated LSTM + E-sharded projection (no cross-core
    communication; the gates matmul is N-bound so only unit-sharding could
    cut PE time, and every measured exchange mechanism costs more than it
    saves). kernel_h24 is the previous 2x4-hybrid fallback."""
    return kernel_rep(x, W_ih, W_hh, b_ih, b_hh, W_out, b_out, _t_steps=_t_steps)

